# revision 22
# baseline (speedup 1.0000x reference)
"""CorrTorch 3D correlation volume kernel for Trainium2 (8 NeuronCores).

Computes, for in1/in2 of shape [1, C=64, D=64, H=128, W=128]:
  out[:, dz*9+dy*3+dx, d, h, w] = mean_c( in1[:,c,d,h,w] * in2p[:,c,d+dz,h+dy,w+dx] )
where in2p is in2 zero-padded by 1 on each side of D/H/W (27 displacements).

Strategy (per core; D sharded 8 ways, halo slabs come from host-padded in2):
  - fp16 inputs, products fp16, channel accumulation fp32 in PSUM.
  - SBUF partitions = (j, c), j = w-half (2*64=128). Both inputs stored in a
    shared padded "slab geometry": 130 rows x 68 cols per w-half, data at
    (h+1, u+2) for in1 and (h', v+2) for in2 (v spans the 66-wide w-halo
    range). Pitch 68 keeps every hot access 4-byte aligned.
  - Products are elementwise; DVE caps at 2 fp16/cycle/lane for 2-input ops,
    so some displacements (dy=1, dx even: flat-capable) run on GpSimd as
    full-slab contiguous ops; the split is tuned so DVE and GpSimd finish
    together. DVE products are emitted in h-halves for tighter PE pipelining.
  - Channel reduction on TensorE (block-ones lhsT, 1/64). All paths use 512-
    row matmuls: compact products via contiguous rhs; GpSimd flat products
    via strided rhs views (8 h-rows x 64 cols at pitch 68), 16 matmuls per
    displacement either way. PSUM chunk map is q-outer so each 32-row PSUM
    group holds a contiguous quarter of the (h,u) output.
  - ScalarE drains PSUM (fp32) to fp16 staging, two displacements per
    staging tile; one DMA per (q, displacement-pair) with contiguous 2048-
    element runs. DMA issue is spread across SP/DVE/Act queues.
"""

import numpy as np
import orjson

import concourse.bass as bass
import concourse.mybir as mybir
import concourse.tile as tile
from concourse.bass_utils import run_bass_kernel_spmd


def _split_multi_waits(bir_json: bytes) -> bytes:
    """TPB instructions encode a single semaphore wait; this walrus build
    refuses instructions with more. Hoist extra waits onto standalone
    EventSemaphore instructions inserted just before, same engine."""
    d = orjson.loads(bir_json)
    for fn in d["functions"]:
        for blk in fn["blocks"]:
            new_insts = []
            for ins in blk["instructions"]:
                si = ins.get("sync_info") or {}
                ow = si.get("on_wait") or []
                if len(ow) > 1:
                    for j, w in enumerate(ow[:-1]):
                        new_insts.append(
                            {
                                "debug": ins.get("debug"),
                                "engine": ins["engine"],
                                "ins": [],
                                "name": f"{ins['name']}_wsplit{j}",
                                "opcode": "EventSemaphore",
                                "outs": [],
                                "sync_info": {"on_wait": [w], "on_update": []},
                            }
                        )
                    si["on_wait"] = [ow[-1]]
                new_insts.append(ins)
            blk["instructions"] = new_insts
    return orjson.dumps(d)

C = 64
D = 64
H = 128
W = 128
NCORES = 8
DSH = D // NCORES  # 8 d-slices per core
NSLAB = DSH + 2  # in2 slabs incl. halo
HP = H + 2  # padded h rows (used by host prep)
TW = W // 2 + 2  # 66 data cols per w-half (with w halo)
PITCH = 68  # slab row pitch (alignment)
SLAB = 130 * PITCH  # 8840 elements per slab per partition
FLAT_LEN = 8704  # 64*136, covers the valid window for all flat deltas
F16 = mybir.dt.float16
F32 = mybir.dt.float32

# Flat-capable displacements (dy=1, dx in {0,2}: delta even, window in range).
GPS_CAPABLE = (3, 5, 12, 14, 21, 23)
# How many of GPS_CAPABLE actually run on GpSimd per d-slice (rest go to
# DVE). Tuned so DVE and GpSimd product time balance.
GPS_COUNT = [5, 5, 5, 5, 5, 5, 5, 5]

_NC_CACHE = None


def _build_nc(nrep=1):
    nc = bass.Bass(target_bir_lowering=False)

    x1 = nc.dram_tensor("x1", [DSH, 128, SLAB], F16, kind="ExternalInput")
    x2 = nc.dram_tensor("x2", [NSLAB, 128, SLAB], F16, kind="ExternalInput")
    # host-shifted copy (x2o[i] = x2[i+1]) keeps the DVE dx=1 windows
    # 4-byte aligned without an on-chip shifted copy.
    x2o = nc.dram_tensor("x2o", [NSLAB, 128, SLAB], F16, kind="ExternalInput")
    # y layout: [d, j, delta, h, u] = [8, 2, 27, 128, 64] fp16
    y = nc.dram_tensor("y", [DSH, 2, 27, H, W // 2], F16, kind="ExternalOutput")

    with tile.TileContext(nc) as tc:
        with (
            tc.tile_pool(name="singles", bufs=1) as singles,
            tc.tile_pool(name="s1p", bufs=2) as s1p,
            tc.tile_pool(name="s2ep", bufs=3) as s2ep,
            tc.tile_pool(name="s2op", bufs=3) as s2op,
            tc.tile_pool(name="prodp", bufs=3) as prodp,
            tc.tile_pool(name="prodg", bufs=2) as prodg,
            tc.tile_pool(name="stagep", bufs=2) as stagep,
            tc.tile_pool(name="psump", bufs=2, space="PSUM") as psump,
        ):
            # Block-ones reduction weights: col m sums partitions
            # [64*(m%2), 64*(m%2)+64), scaled by 1/C to fold in the channel
            # mean. 16 duplicated column-pairs so each matmul fills a full
            # 32-partition col-group of PSUM (dense PSUM -> cheap drain copy).
            ones = singles.tile([128, 32], F16)
            nc.vector.memset(ones[:], 0.0)
            ones_v = ones[:].rearrange("p (v m) -> p v m", m=2)
            nc.vector.memset(ones_v[0:64, :, 0:1], 1.0 / C)
            nc.vector.memset(ones_v[64:128, :, 1:2], 1.0 / C)

            s1_tiles = {}
            s2e_tiles = {}
            s2o_tiles = {}

            def load_s1(d):
                # 4 quarter-DMAs: the load starts late (waits for the s1 pool
                # slot) and sits on the next d's critical path.
                t = s1p.tile([128, SLAB], F16, tag="s1")
                qtr = SLAB // 4
                for i in range(4):
                    lo, hi = i * qtr, SLAB if i == 3 else (i + 1) * qtr
                    nc.sync.dma_start(out=t[:, lo:hi], in_=x1[d][:, lo:hi])
                s1_tiles[d] = t

            def load_s2(dl):
                t = s2ep.tile([128, SLAB], F16, tag="s2e")
                half = SLAB // 2
                nc.sync.dma_start(out=t[:, 0:half], in_=x2[dl][:, 0:half])
                nc.sync.dma_start(out=t[:, half:], in_=x2[dl][:, half:])
                s2e_tiles[dl] = t
                o = s2op.tile([128, SLAB], F16, tag="s2o")
                nc.sync.dma_start(out=o[:, 0:half], in_=x2o[dl][:, 0:half])
                nc.sync.dma_start(out=o[:, half:], in_=x2o[dl][:, half:])
                s2o_tiles[dl] = o

            # Output DMAs mostly issue from SP; q2 on Act spreads the DGE
            # config load across both HWDGE-capable sequencers.
            out_dma_engines = [nc.sync, nc.sync, nc.scalar, nc.sync]

            for _rep in range(nrep):
              s1_tiles.clear()
              s2e_tiles.clear()
              s2o_tiles.clear()
              load_s2(0)
              load_s1(0)
              for dl in (1, 2):
                  load_s2(dl)

              for d in range(DSH):
                if d + 1 < DSH:
                    load_s1(d + 1)
                t1 = s1_tiles[d]
                t1v = t1[:].rearrange("p (r t) -> p r t", t=PITCH)[:, 1:129, 2:66]

                gps_set = set(GPS_CAPABLE[: GPS_COUNT[d]])
                # Emission order: spread the slow GpSimd (flat) displacements
                # evenly so the PE never waits ~17us on a flat product while
                # DVE-fed work is available. Flat slots every ~5 positions.
                g = len(gps_set)
                flat_slots = {2, 7, 12, 17, 22} if g == 5 else {2, 6, 10, 14, 18, 22}
                dve_dis = [i for i in range(27) if i not in gps_set]
                flat_dis = sorted(gps_set)
                order = []
                it_d, it_f = iter(dve_dis), iter(flat_dis)
                for slot in range(27):
                    order.append(next(it_f) if slot in flat_slots else next(it_d))
                gps_list = [di for di in order if di in gps_set]
                flat_tiles = {}

                def flat_product(di):
                    # flat full-slab product on GpSimd: covers all (h, u) at
                    # pitch 68 plus halo junk that the matmuls never read.
                    dz, rem = divmod(di, 9)
                    dy, dx = divmod(rem, 3)
                    delta = PITCH * (dy - 1) + dx
                    x0 = 68
                    pg = prodg.tile([128, FLAT_LEN], F16, tag="prodg")
                    nc.gpsimd.tensor_mul(
                        pg[:],
                        t1[:, x0 : x0 + FLAT_LEN],
                        s2e_tiles[d + dz][:, x0 + delta : x0 + delta + FLAT_LEN],
                    )
                    flat_tiles[di] = pg

                # Issue the first two flat products up front so GpSimd runs
                # ahead of the PE consumption point (prodg pool: 2 bufs).
                n_issued = min(2, len(gps_list))
                for di in gps_list[:n_issued]:
                    flat_product(di)

                # s2 slab d+3 can only load once slab d's pool slot frees:
                # after the last dz=0 displacement in emission order.
                last_dz0_pos = max(p for p, v in enumerate(order) if v < 9)

                for pos in range(27):
                    di = order[pos]
                    dz, rem = divmod(di, 9)
                    dy, dx = divmod(rem, 3)
                    if pos == last_dz0_pos + 1 and d + 3 < NSLAB:
                        load_s2(d + 3)
                    ps = psump.tile([128, 2048], F32, tag="ps")
                    if di in gps_set:
                        pg = flat_tiles.pop(di)
                        # refill the GpSimd pipeline with the next flat product
                        if n_issued < len(gps_list):
                            flat_product(gps_list[n_issued])
                            n_issued += 1
                        pg3 = pg[:].rearrange("p (r t) -> p r t", t=PITCH)
                        for i16 in range(16):
                            # cycle tile positions (q) every matmul; PSUM
                            # destinations unchanged (q-outer chunk map)
                            c16 = (i16 % 4) * 4 + i16 // 4
                            q, s = divmod(c16, 4)
                            nc.tensor.matmul(
                                ps[32 * q : 32 * q + 32, 512 * s : 512 * s + 512],
                                lhsT=ones[:],
                                rhs=pg3[:, 8 * c16 : 8 * c16 + 8, 2:66],
                                start=True,
                                stop=True,
                                tile_position=(0, 32 * q),
                            )
                    else:
                        if dx == 1:
                            t2v = s2o_tiles[d + dz][:].rearrange(
                                "p (r t) -> p r t", t=PITCH
                            )[:, dy : dy + H, 2:66]
                        else:
                            t2v = s2e_tiles[d + dz][:].rearrange(
                                "p (r t) -> p r t", t=PITCH
                            )[:, dy : dy + H, dx + 2 : dx + 66]
                        # two h-halves for finer DVE->PE pipelining
                        for half in range(2):
                            hs = 64 * half
                            p = prodp.tile([128, 64, W // 2], F16, tag="prod")
                            nc.vector.tensor_mul(
                                p[:], t1v[:, hs : hs + 64, :], t2v[:, hs : hs + 64, :]
                            )
                            pf = p[:].rearrange("p h u -> p (h u)")
                            for i in range(8):
                                # cycle positions: half 0 -> q 0,1; half 1 -> q 2,3
                                c16 = 8 * half + (i % 2) * 4 + i // 2
                                q, s = divmod(c16, 4)
                                li = (c16 - 8 * half)
                                nc.tensor.matmul(
                                    ps[32 * q : 32 * q + 32, 512 * s : 512 * s + 512],
                                    lhsT=ones[:],
                                    rhs=pf[:, 512 * li : 512 * li + 512],
                                    start=True,
                                    stop=True,
                                    tile_position=(0, 32 * q),
                                )
                    # drain PSUM (fp32) to fp16 staging, then 4 DMAs (one per
                    # 32-row PSUM group q; rows 32q,32q+1 hold the two j
                    # halves of output chunk range 4q..4q+3, contiguous 2048).
                    st = stagep.tile([128, 2048], F16, tag="stage")
                    nc.scalar.copy(out=st[:], in_=ps[:])
                    for q in range(4):
                        src = st[32 * q : 32 * q + 2, :]
                        dst = bass.AP(
                            y,
                            d * 442368 + di * 8192 + q * 2048,
                            [[221184, 2], [1, 2048]],
                        )
                        out_dma_engines[q].dma_start(out=dst, in_=src)

    _orig_to_json = nc.to_json_bytes
    nc.to_json_bytes = lambda: _split_multi_waits(_orig_to_json())
    return nc


def _prep_core_inputs(a1, a2p, k):
    # a1: [C, D, H, W] fp16 ; a2p: [C, D+2, HP, 130] fp16 (host-padded)
    s1 = a1[:, 8 * k : 8 * k + DSH]  # [C, 8, H, W]
    # [d, j, c, h, u]
    s1 = s1.reshape(C, DSH, H, 2, W // 2).transpose(1, 3, 0, 2, 4)
    x1 = np.zeros((DSH, 128, 130, PITCH), np.float16)
    x1[:, :, 1 : H + 1, 2:66] = s1.reshape(DSH, 128, H, W // 2)
    x1 = x1.reshape(DSH, 128, SLAB)

    b = a2p[:, 8 * k : 8 * k + NSLAB]  # [C, 10, HP, 130]
    s2 = np.stack([b[..., 0:TW], b[..., W // 2 : W // 2 + TW]], axis=0)
    # [j, C, 10, HP, TW] -> [d', j, c, h_p, v]
    s2 = np.ascontiguousarray(s2.transpose(2, 0, 1, 3, 4)).reshape(
        NSLAB, 128, HP, TW
    )
    x2 = np.zeros((NSLAB, 128, 130, PITCH), np.float16)
    x2[:, :, 0:HP, 2 : 2 + TW] = s2
    x2 = x2.reshape(NSLAB, 128, SLAB)
    x2o = np.zeros_like(x2)
    x2o[:, :, : SLAB - 1] = x2[:, :, 1:]
    return {"x1": x1, "x2": x2, "x2o": x2o}


def kernel(in1, in2):
    global _NC_CACHE
    in1 = np.asarray(in1)[0]  # [C, D, H, W] f32
    in2 = np.asarray(in2)[0]

    a1 = in1.astype(np.float16)
    a2p = np.zeros((C, D + 2, HP, W + 2), np.float16)
    a2p[:, 1 : D + 1, 1 : H + 1, 1 : W + 1] = in2

    in_maps = [_prep_core_inputs(a1, a2p, k) for k in range(NCORES)]

    if _NC_CACHE is None:
        _NC_CACHE = _build_nc()
    results = run_bass_kernel_spmd(_NC_CACHE, in_maps, core_ids=list(range(NCORES)))

    # per-core y: [DSH, 2, 27, H, 64] -> [27, DSH, H, W]
    parts = [
        r["y"].astype(np.float32).transpose(2, 0, 3, 1, 4).reshape(27, DSH, H, W)
        for r in results.results
    ]
    out = np.concatenate(parts, axis=1).reshape(1, 27, D, H, W)
    return np.ascontiguousarray(out)
